# revision 2
# baseline (speedup 1.0000x reference)
"""ChebConv SpMM kernel for 8 TRN2 NeuronCores — matmul-aggregation version.

Strategy (dest-sharded graph-parallel):
- x held as [V, 128] bf16 (col = b*32 + fi). Core c owns dest rows
  [c*Vc, (c+1)*Vc). Full x is AllGathered (bf16, half-shard granularity
  for overlap) into per-step gather buffers xfA/xfB with row order
  (core, half-local row).
- Each Chebyshev step, per dest supertile of 16 dest tiles (128 dests
  each): dma_gather the edges' source rows (256B bf16 elements) from the
  6 int16-index regions, build a scaled one-hot matrix
  S[e, d] = val_e * (dest_e == d) on DVE (is_equal vs a host-provided
  iota row matrix, then multiply by vals), and accumulate
  y_tile = sum_chunks S^T-matmul-E into PSUM on the PE. The recurrence
  term -x_{k-1} is folded in as one extra matmul with a -I stationary.
  No dma_scatter_add anywhere.
- Edges are host-sorted into (dest supertile, source region, dest tile)
  order; each (tile, region) group is padded to a multiple of 128 edges
  (val=0 pads) so every matmul chunk is a full 128-edge slab.
- Final projection out = sum_k T_k W_k: PE transpose of bf16 cheb tiles
  + matmul against block-diagonal bf16 W, bias added on the PSUM->SBUF
  copy (f32 out).
"""
import sys

sys.path.insert(0, "/opt/trn_rl_repo")

import numpy as np
from ml_dtypes import bfloat16

import concourse.bass as bass
import concourse.bacc as bacc
import concourse.mybir as mybir
import concourse.tile as tile
from concourse import bass_utils

F32 = mybir.dt.float32
BF16 = mybir.dt.bfloat16
I16 = mybir.dt.int16

DEBUG_DUMP = False

V = 196608
C = 8
VC = V // C            # 24576 dest rows per core
HALF = VC // 2         # 12288
NT = VC // 128         # 192 dest tiles per core
ST_T = 8               # dest tiles per supertile (= PSUM banks)
NST = NT // ST_T       # 12 supertiles
RG = 32768             # gather region rows (int16 index range)
NREG = 6               # regions over the 2x[98304,128] gather buffers
B, FIN, FOUT, K = 4, 32, 64, 5
CW = B * FIN           # 128
BFO = B * FOUT         # 256
KS = K - 1             # SpMM steps


# ----------------------------------------------------------------------------
# Host-side preprocessing
# ----------------------------------------------------------------------------

class Plan:
    pass


def build_plan(lap_rows, lap_cols, lap_vals):
    rows = np.asarray(lap_rows).astype(np.int64)
    cols = np.asarray(lap_cols).astype(np.int64)
    vals = np.asarray(lap_vals).astype(np.float32)
    nnz = len(rows)

    core = rows // VC
    tloc = (rows % VC) // 128          # dest tile within core
    dest = rows % 128                  # dest within tile
    csrc = cols // VC
    j = cols % VC
    # gather buffer row: xfA holds lo half-shards (rows c*HALF + j),
    # xfB the hi half-shards; region = 32768-row slice for int16 indices
    buf = (j >= HALF).astype(np.int64)
    brow = csrc * HALF + (j % HALF)
    reg = buf * 3 + brow // RG         # 0..5
    gidx = brow % RG

    gk = (core * NT + tloc) * NREG + reg
    order = np.argsort(gk, kind="stable")
    gs = gk[order]
    starts = np.r_[0, np.nonzero(np.diff(gs))[0] + 1]
    grp_start = np.repeat(starts, np.diff(np.r_[starts, nnz]))
    within = np.arange(nnz) - grp_start

    cnt = np.bincount(gk, minlength=C * NT * NREG).reshape(C, NT, NREG)
    mx = cnt.max(axis=0)               # [NT, NREG]
    assert cnt.sum(axis=(0, 2)).min() > 0
    chunks = (mx + 127) // 128         # [NT, NREG]

    # slot layout: (supertile, region, tile) order
    off = np.zeros((NT, NREG), np.int64)
    cursor = 0
    st_groups = []                     # [NST][NREG] -> (o0, n)
    st_mm = []                         # [NST][NREG] -> list of (ti, ci, first)
    st_last = []                       # [NST] -> per-ti (r, ci) of last chunk
    for st in range(NST):
        groups_r = []
        mm_r = []
        tot_ch = chunks[st * ST_T:(st + 1) * ST_T].sum(axis=1)
        done = np.zeros(ST_T, np.int64)
        last = [None] * ST_T
        for r in range(NREG):
            o0 = cursor
            mm = []
            ci = 0
            for ti in range(ST_T):
                t = st * ST_T + ti
                off[t, r] = cursor
                nch = int(chunks[t, r])
                for _ in range(nch):
                    mm.append((ti, ci, done[ti] == 0))
                    done[ti] += 1
                    if done[ti] == tot_ch[ti]:
                        last[ti] = (r, ci)
                    ci += 1
                cursor += nch * 128
            groups_r.append((o0, cursor - o0))
            mm_r.append(mm)
        assert all(l is not None for l in last)
        st_groups.append(groups_r)
        st_mm.append(mm_r)
        st_last.append(last)
    TOT = cursor
    assert TOT % 128 == 0

    t_s = tloc[order]
    r_s = reg[order]
    c_s = core[order]
    slot_s = off[t_s, r_s] + within

    g_arr = np.zeros((C, TOT), np.int16)
    d_arr = np.zeros((C, TOT), np.float32)
    v_arr = np.zeros((C, TOT), np.float32)
    g_arr[c_s, slot_s] = gidx[order].astype(np.int16)
    d_arr[c_s, slot_s] = dest[order]
    v_arr[c_s, slot_s] = vals[order]

    def wrap16(a):                     # [C, TOT] -> [C, 128, TOT//16]
        w = a.reshape(C, TOT // 16, 16).transpose(0, 2, 1)
        return np.tile(w, (1, 8, 1)).copy()

    def wrap128(a):
        return a.reshape(C, TOT // 128, 128).transpose(0, 2, 1).copy()

    plan = Plan()
    plan.TOT = TOT
    plan.st_groups = st_groups
    plan.st_mm = st_mm
    plan.st_last = st_last
    plan.NGMAX = max(n // 128 for g in st_groups for (_, n) in g)
    plan.NST16 = max(sum(n for (_, n) in g) // 16 for g in st_groups)
    plan.NST128 = max(sum(n for (_, n) in g) // 128 for g in st_groups)
    plan.gidx = wrap16(g_arr)
    plan.dest = wrap128(d_arr).astype(bfloat16)
    plan.gvals = wrap128(v_arr).astype(bfloat16)
    plan.gvals2 = wrap128(2.0 * v_arr).astype(bfloat16)
    return plan


def host_prep(lap_rows, lap_cols, lap_vals, inputs, weight, bias):
    plan = build_plan(lap_rows, lap_cols, lap_vals)
    # x0 [V, 128], col = b*FIN + fi
    x0 = np.ascontiguousarray(
        np.asarray(inputs, np.float32).transpose(1, 0, 2).reshape(V, CW)
    ).astype(bfloat16)
    # Reference pairs cheb col (fi, k) with weight.reshape(K*Fin, F)[fi*K + k]
    W_eff = np.asarray(weight, np.float32).reshape(K * FIN, FOUT) \
        .reshape(FIN, K, FOUT).transpose(1, 0, 2)
    Wblk = np.zeros((K, CW, BFO), np.float32)
    for k in range(K):
        for b in range(B):
            Wblk[k, b * FIN:(b + 1) * FIN, b * FOUT:(b + 1) * FOUT] = W_eff[k]
    Wblk = Wblk.astype(bfloat16)
    bias_row = np.tile(np.tile(np.asarray(bias, np.float32), B)[None, :],
                       (128, 1))
    ident = np.eye(128, dtype=np.float32).astype(bfloat16)
    negi = (-np.eye(128, dtype=np.float32)).astype(bfloat16)
    iorow = np.tile(np.arange(128, dtype=np.float32)[None, :],
                    (128, 1)).astype(bfloat16)
    in_maps = []
    for c in range(C):
        in_maps.append({
            "x0s": x0[c * VC:(c + 1) * VC],
            "gidx": plan.gidx[c], "dest": plan.dest[c],
            "gvals": plan.gvals[c], "gvals2": plan.gvals2[c],
            "wblk": Wblk, "bias_row": bias_row,
            "iden": ident, "negi": negi, "iorow": iorow,
        })
    return plan, in_maps


# ----------------------------------------------------------------------------
# Device program
# ----------------------------------------------------------------------------

def build_program(plan):
    TOT = plan.TOT
    NGMAX = plan.NGMAX

    nc = bacc.Bacc("TRN2", target_bir_lowering=False, debug=False,
                   num_devices=C)
    x0s = nc.dram_tensor("x0s", [VC, CW], BF16, kind="ExternalInput")
    gidx = nc.dram_tensor("gidx", [128, TOT // 16], I16, kind="ExternalInput")
    dest = nc.dram_tensor("dest", [128, TOT // 128], BF16,
                          kind="ExternalInput")
    gvals = nc.dram_tensor("gvals", [128, TOT // 128], BF16,
                           kind="ExternalInput")
    gvals2 = nc.dram_tensor("gvals2", [128, TOT // 128], BF16,
                            kind="ExternalInput")
    wblk = nc.dram_tensor("wblk", [K, CW, BFO], BF16, kind="ExternalInput")
    bias_row = nc.dram_tensor("bias_row", [128, BFO], F32,
                              kind="ExternalInput")
    iden = nc.dram_tensor("iden", [128, 128], BF16, kind="ExternalInput")
    negi = nc.dram_tensor("negi", [128, 128], BF16, kind="ExternalInput")
    iorow = nc.dram_tensor("iorow", [128, 128], BF16, kind="ExternalInput")
    out = nc.dram_tensor("out", [VC, BFO], F32, kind="ExternalOutput")
    dbg = nc.dram_tensor("dbg", [512, 128], F32, kind="ExternalOutput") \
        if DEBUG_DUMP else None

    # internal DRAM
    x0b = [nc.dram_tensor(f"x0b{h}", [HALF, CW], BF16) for h in range(2)]
    xf = [[nc.dram_tensor(f"xf{s}_{h}", [HALF * C, CW], BF16,
                          addr_space="Shared")
           for h in range(2)] for s in range(KS)]
    yb = [[nc.dram_tensor(f"y{s}_{h}", [HALF, CW], BF16) for h in range(2)]
          for s in range(1, KS + 1)]        # yb[s-1] = x_s shard halves

    def shard_rows(s, r0, r1):
        """AP for local shard rows [r0, r1) of x_s (s=0 -> x0b)."""
        src = x0b if s == 0 else yb[s - 1]
        assert (r0 < HALF) == (r1 <= HALF)
        if r1 <= HALF:
            return src[0][r0:r1, :]
        return src[1][r0 - HALF:r1 - HALF, :]

    rg_list = [[0, 1, 2, 3, 4, 5, 6, 7]]

    with tile.TileContext(nc) as tc:
        with tc.tile_pool(name="cns", bufs=1) as cns:
            iot = cns.tile([128, 128], BF16, tag="iot")
            nc.sync.dma_start(iot[:], iorow[:])
            ngt = cns.tile([128, 128], BF16, tag="ngt")
            nc.sync.dma_start(ngt[:], negi[:])
            # bounce x0 shard into internal halves, AllGather into xf[0]
            for h in range(2):
                nc.sync.dma_start(x0b[h][:, :],
                                  x0s[h * HALF:(h + 1) * HALF, :])
                nc.gpsimd.collective_compute(
                    "AllGather", mybir.AluOpType.bypass,
                    replica_groups=rg_list,
                    ins=[x0b[h].ap().opt()],
                    outs=[xf[0][h].ap().opt()])

            with tc.tile_pool(name="gip", bufs=2) as gip, \
                 tc.tile_pool(name="dtp", bufs=2) as dtp, \
                 tc.tile_pool(name="vtp", bufs=2) as vtp, \
                 tc.tile_pool(name="rng", bufs=3) as rng, \
                 tc.tile_pool(name="smp", bufs=3) as smp, \
                 tc.tile_pool(name="ysp", bufs=2) as ysp, \
                 tc.tile_pool(name="xpp", bufs=2) as xpp, \
                 tc.tile_pool(name="psp", bufs=1, space="PSUM") as psp:
                for s in range(KS):
                    vsrc = gvals if s == 0 else gvals2
                    for st in range(NST):
                        st_o0 = plan.st_groups[st][0][0]
                        st_n = sum(n for (_, n) in plan.st_groups[st])
                        # per-ST index/dest/val loads (one DMA each)
                        gi = gip.tile([128, plan.NST16], I16, tag="gi")
                        nc.sync.dma_start(
                            gi[:, :st_n // 16],
                            gidx[:, st_o0 // 16:(st_o0 + st_n) // 16])
                        dt = dtp.tile([128, plan.NST128], BF16, tag="dt")
                        nc.sync.dma_start(
                            dt[:, :st_n // 128],
                            dest[:, st_o0 // 128:(st_o0 + st_n) // 128])
                        vt = vtp.tile([128, plan.NST128], BF16, tag="vt")
                        nc.sync.dma_start(
                            vt[:, :st_n // 128],
                            vsrc[:, st_o0 // 128:(st_o0 + st_n) // 128])
                        ps = [psp.tile([128, 128], F32, tag=f"ps{q}",
                                       name=f"ps{q}") for q in range(ST_T)]
                        for r in range(NREG):
                            o0, n = plan.st_groups[st][r]
                            if n == 0:
                                continue
                            ng = n // 128
                            lo16 = (o0 - st_o0) // 16
                            lo128 = (o0 - st_o0) // 128
                            rg = rng.tile([128, NGMAX, CW], BF16, tag="rg")
                            src = xf[s][r // 3][(r % 3) * RG:(r % 3 + 1) * RG, :]
                            for p0 in range(0, n, 1024):
                                pn = min(1024, n - p0)
                                nc.gpsimd.dma_gather(
                                    rg[:, p0 // 128:(p0 + pn) // 128, :], src,
                                    gi[:, lo16 + p0 // 16:
                                       lo16 + (p0 + pn) // 16],
                                    pn, pn, CW)
                            sm = smp.tile([128, NGMAX, CW], BF16, tag="sm")
                            nc.vector.tensor_tensor(
                                sm[:, :ng, :],
                                dt[:, lo128:lo128 + ng].unsqueeze(-1)
                                .broadcast_to([128, ng, CW]),
                                iot[:].unsqueeze(1).broadcast_to([128, ng, CW]),
                                mybir.AluOpType.is_equal)
                            nc.vector.tensor_tensor(
                                sm[:, :ng, :], sm[:, :ng, :],
                                vt[:, lo128:lo128 + ng].unsqueeze(-1)
                                .broadcast_to([128, ng, CW]),
                                mybir.AluOpType.mult)
                            if DEBUG_DUMP and s == 0 and st == 0 and r == 0:
                                dts = smp.tile([128, 128], F32, tag="dts",
                                               name="dts")
                                nc.vector.tensor_copy(dts[:], sm[:, 0, :])
                                nc.sync.dma_start(dbg[0:128, :], dts[:])
                                dtr = smp.tile([128, 128], F32, tag="dtr",
                                               name="dtr")
                                nc.vector.tensor_copy(dtr[:], rg[:, 0, :])
                                nc.sync.dma_start(dbg[128:256, :], dtr[:])
                            for (ti, ci, first) in plan.st_mm[st][r]:
                                oap = ps[ti][:, :]
                                stop = (s == 0 and
                                        plan.st_last[st][ti] == (r, ci))
                                nc.tensor.matmul(
                                    oap, sm[:, ci, :], rg[:, ci, :],
                                    start=bool(first), stop=stop)
                        ys = ysp.tile([128, ST_T, CW], BF16, tag="ys")
                        if s > 0:
                            xp = xpp.tile([128, ST_T, CW], BF16, tag="xp")
                            nc.sync.dma_start(
                                xp[:],
                                shard_rows(s - 1, st * 1024, (st + 1) * 1024)
                                .rearrange("(ts p) c -> p ts c", p=128))
                            for ti in range(ST_T):
                                nc.tensor.matmul(ps[ti][:, :], ngt[:],
                                                 xp[:, ti, :],
                                                 start=False, stop=True)
                        for ti in range(ST_T):
                            nc.vector.tensor_copy(ys[:, ti, :], ps[ti][:, :])
                        if DEBUG_DUMP and s == 0 and st == 0:
                            dty = ysp.tile([128, 128], F32, tag="dty",
                                           name="dty")
                            nc.vector.tensor_copy(dty[:], ys[:, 0, :])
                            nc.sync.dma_start(dbg[256:384, :], dty[:])
                        if DEBUG_DUMP and s == 1 and st == 0:
                            dtz = ysp.tile([128, 128], F32, tag="dtz",
                                           name="dtz")
                            nc.vector.tensor_copy(dtz[:], ys[:, 0, :])
                            nc.sync.dma_start(dbg[384:512, :], dtz[:])
                        h, hb = (0, st * 1024) if st < NST // 2 else \
                            (1, st * 1024 - HALF)
                        nc.sync.dma_start(
                            yb[s][h][hb:hb + 1024, :]
                            .rearrange("(ts p) c -> p ts c", p=128), ys[:])
                        if st == NST // 2 - 1 and s < KS - 1:
                            nc.gpsimd.collective_compute(
                                "AllGather", mybir.AluOpType.bypass,
                                replica_groups=rg_list,
                                ins=[yb[s][0].ap().opt()],
                                outs=[xf[s + 1][0].ap().opt()])
                        if st == NST - 1 and s < KS - 1:
                            nc.gpsimd.collective_compute(
                                "AllGather", mybir.AluOpType.bypass,
                                replica_groups=rg_list,
                                ins=[yb[s][1].ap().opt()],
                                outs=[xf[s + 1][1].ap().opt()])

        # ---- projection ----
        VSUP = 2048
        NSUP = VC // VSUP
        TSUB = VSUP // 128
        with tc.tile_pool(name="cwp", bufs=1) as cwp, \
             tc.tile_pool(name="cbp", bufs=2) as cbp, \
             tc.tile_pool(name="pst", bufs=2, space="PSUM") as pst, \
             tc.tile_pool(name="pso", bufs=2, space="PSUM") as pso, \
             tc.tile_pool(name="lhp", bufs=3) as lhp, \
             tc.tile_pool(name="stp", bufs=2) as stp:
            wt = []
            for k in range(K):
                wtk = cwp.tile([CW, BFO], BF16, tag=f"w{k}")
                wt.append(wtk)
            for k in range(K):
                nc.sync.dma_start(wt[k][:], wblk[k, :, :])
            bt = cwp.tile([128, BFO], F32, tag="bias")
            nc.sync.dma_start(bt[:], bias_row[:])
            idt = cwp.tile([128, 128], BF16, tag="ident")
            nc.sync.dma_start(idt[:], iden[:])
            for sc in range(NSUP):
                vbase = sc * VSUP
                cb = []
                for k in range(K):
                    cbt = cbp.tile([128, TSUB, CW], BF16, tag=f"cb{k}")
                    nc.sync.dma_start(
                        cbt[:],
                        shard_rows(k, vbase, vbase + VSUP)
                        .rearrange("(ts p) c -> p ts c", p=128))
                    cb.append(cbt)
                stl = stp.tile([128, TSUB, BFO], F32, tag="stage")
                for ts in range(TSUB):
                    po = pso.tile([128, BFO], F32, tag="po")
                    for k in range(K):
                        pt = pst.tile([128, 128], BF16, tag="pt")
                        nc.tensor.transpose(pt[:, :CW], cb[k][:, ts, :],
                                            idt[:])
                        lh = lhp.tile([128, CW], BF16, tag="lh")
                        nc.vector.tensor_copy(lh[:, :CW], pt[:, :CW])
                        nc.tensor.matmul(po[:], lh[:, :CW], wt[k][:],
                                         start=(k == 0), stop=(k == K - 1))
                    nc.vector.tensor_tensor(
                        stl[:, ts, :], po[:], bt[:], mybir.AluOpType.add)
                nc.sync.dma_start(
                    out[vbase:vbase + VSUP, :]
                    .rearrange("(ts p) c -> p ts c", p=128), stl[:])
    nc.compile()
    return nc


# ----------------------------------------------------------------------------
# Self-contained entry point (full inputs in, full output out)
# ----------------------------------------------------------------------------

_CACHE = {}


def kernel(lap_rows, lap_cols, lap_vals, inputs, weight, bias):
    """ChebConv on 8 TRN2 NeuronCores. Full inputs -> full [B, V, FOUT]."""
    inputs = np.asarray(inputs, np.float32)
    weight = np.asarray(weight, np.float32)
    bias = np.asarray(bias, np.float32)
    plan, in_maps = host_prep(lap_rows, lap_cols, lap_vals, inputs, weight,
                              bias)
    key = ("v2", plan.TOT)
    if key not in _CACHE:
        _CACHE.clear()
        _CACHE[key] = build_program(plan)
    nc = _CACHE[key]
    res = bass_utils.run_bass_kernel_spmd(nc, in_maps, core_ids=list(range(C)))
    outs = []
    for c in range(C):
        o = res.results[c]["out"]
        outs.append(o.reshape(VC, B, FOUT).transpose(1, 0, 2))
    return np.ascontiguousarray(np.concatenate(outs, axis=1)).astype(np.float32)


# revision 3
# speedup vs baseline: 1.0405x; 1.0405x over previous
"""ChebConv SpMM kernel for 8 TRN2 NeuronCores — matmul-aggregation version.

Strategy (dest-sharded graph-parallel):
- x held as [V, 128] bf16 (col = b*32 + fi). Core c owns dest rows
  [c*Vc, (c+1)*Vc). Full x is AllGathered (bf16, half-shard granularity
  for overlap) into per-step gather buffers xfA/xfB with row order
  (core, half-local row).
- Each Chebyshev step, per dest supertile of 16 dest tiles (128 dests
  each): dma_gather the edges' source rows (256B bf16 elements) from the
  6 int16-index regions, build a scaled one-hot matrix
  S[e, d] = val_e * (dest_e == d) on DVE (is_equal vs a host-provided
  iota row matrix, then multiply by vals), and accumulate
  y_tile = sum_chunks S^T-matmul-E into PSUM on the PE. The recurrence
  term -x_{k-1} is folded in as one extra matmul with a -I stationary.
  No dma_scatter_add anywhere.
- Edges are host-sorted into (dest supertile, source region, dest tile)
  order; each (tile, region) group is padded to a multiple of 128 edges
  (val=0 pads) so every matmul chunk is a full 128-edge slab.
- Final projection out = sum_k T_k W_k: PE transpose of bf16 cheb tiles
  + matmul against block-diagonal bf16 W, bias added on the PSUM->SBUF
  copy (f32 out).
"""
import sys

sys.path.insert(0, "/opt/trn_rl_repo")

import numpy as np
from ml_dtypes import bfloat16

import concourse.bass as bass
import concourse.bacc as bacc
import concourse.mybir as mybir
import concourse.tile as tile
from concourse import bass_utils

F32 = mybir.dt.float32
BF16 = mybir.dt.bfloat16
I16 = mybir.dt.int16

DEBUG_DUMP = False
CALL_CAP = 1024        # idxs per dma_gather call (<=2048: 128-deep desc ring)

V = 196608
C = 8
VC = V // C            # 24576 dest rows per core
HALF = VC // 2         # 12288
NT = VC // 128         # 192 dest tiles per core
ST_T = 8               # dest tiles per supertile (= PSUM banks)
NST = NT // ST_T       # 12 supertiles
RG = 32768             # gather region rows (int16 index range)
NREG = 6               # regions over the 2x[98304,128] gather buffers
B, FIN, FOUT, K = 4, 32, 64, 5
CW = B * FIN           # 128
BFO = B * FOUT         # 256
KS = K - 1             # SpMM steps


# ----------------------------------------------------------------------------
# Host-side preprocessing
# ----------------------------------------------------------------------------

class Plan:
    pass


def build_plan(lap_rows, lap_cols, lap_vals):
    rows = np.asarray(lap_rows).astype(np.int64)
    cols = np.asarray(lap_cols).astype(np.int64)
    vals = np.asarray(lap_vals).astype(np.float32)
    nnz = len(rows)

    core = rows // VC
    tloc = (rows % VC) // 128          # dest tile within core
    dest = rows % 128                  # dest within tile
    csrc = cols // VC
    j = cols % VC
    # gather buffer row: xfA holds lo half-shards (rows c*HALF + j),
    # xfB the hi half-shards; region = 32768-row slice for int16 indices
    buf = (j >= HALF).astype(np.int64)
    brow = csrc * HALF + (j % HALF)
    reg = buf * 3 + brow // RG         # 0..5
    gidx = brow % RG

    gk = (core * NT + tloc) * NREG + reg
    order = np.argsort(gk, kind="stable")
    gs = gk[order]
    starts = np.r_[0, np.nonzero(np.diff(gs))[0] + 1]
    grp_start = np.repeat(starts, np.diff(np.r_[starts, nnz]))
    within = np.arange(nnz) - grp_start

    cnt = np.bincount(gk, minlength=C * NT * NREG).reshape(C, NT, NREG)
    mx = cnt.max(axis=0)               # [NT, NREG]
    assert cnt.sum(axis=(0, 2)).min() > 0
    chunks = (mx + 127) // 128         # [NT, NREG]

    # slot layout: (supertile, region, tile) order
    off = np.zeros((NT, NREG), np.int64)
    cursor = 0
    st_groups = []                     # [NST][NREG] -> (o0, n)
    st_mm = []                         # [NST][NREG] -> list of (ti, ci, first)
    st_last = []                       # [NST] -> per-ti (r, ci) of last chunk
    for st in range(NST):
        groups_r = []
        mm_r = []
        tot_ch = chunks[st * ST_T:(st + 1) * ST_T].sum(axis=1)
        done = np.zeros(ST_T, np.int64)
        last = [None] * ST_T
        for r in range(NREG):
            o0 = cursor
            mm = []
            ci = 0
            for ti in range(ST_T):
                t = st * ST_T + ti
                off[t, r] = cursor
                nch = int(chunks[t, r])
                for _ in range(nch):
                    mm.append((ti, ci, done[ti] == 0))
                    done[ti] += 1
                    if done[ti] == tot_ch[ti]:
                        last[ti] = (r, ci)
                    ci += 1
                cursor += nch * 128
            groups_r.append((o0, cursor - o0))
            mm_r.append(mm)
        assert all(l is not None for l in last)
        st_groups.append(groups_r)
        st_mm.append(mm_r)
        st_last.append(last)
    TOT = cursor
    assert TOT % 128 == 0

    t_s = tloc[order]
    r_s = reg[order]
    c_s = core[order]
    slot_s = off[t_s, r_s] + within

    g_arr = np.zeros((C, TOT), np.int16)
    d_arr = np.zeros((C, TOT), np.float32)
    v_arr = np.zeros((C, TOT), np.float32)
    g_arr[c_s, slot_s] = gidx[order].astype(np.int16)
    d_arr[c_s, slot_s] = dest[order]
    v_arr[c_s, slot_s] = vals[order]

    def wrap16(a):                     # [C, TOT] -> [C, 128, TOT//16]
        w = a.reshape(C, TOT // 16, 16).transpose(0, 2, 1)
        return np.tile(w, (1, 8, 1)).copy()

    def wrap128(a):
        return a.reshape(C, TOT // 128, 128).transpose(0, 2, 1).copy()

    plan = Plan()
    plan.TOT = TOT
    plan.st_groups = st_groups
    plan.st_mm = st_mm
    plan.st_last = st_last
    plan.NGMAX = max(n // 128 for g in st_groups for (_, n) in g)
    plan.NST16 = max(sum(n for (_, n) in g) // 16 for g in st_groups)
    plan.NST128 = max(sum(n for (_, n) in g) // 128 for g in st_groups)
    plan.gidx = wrap16(g_arr)
    plan.dest = wrap128(d_arr).astype(bfloat16)
    plan.gvals = wrap128(v_arr).astype(bfloat16)
    plan.gvals2 = wrap128(2.0 * v_arr).astype(bfloat16)
    return plan


def host_prep(lap_rows, lap_cols, lap_vals, inputs, weight, bias):
    plan = build_plan(lap_rows, lap_cols, lap_vals)
    # x0 [V, 128], col = b*FIN + fi
    x0 = np.ascontiguousarray(
        np.asarray(inputs, np.float32).transpose(1, 0, 2).reshape(V, CW)
    ).astype(bfloat16)
    # Reference pairs cheb col (fi, k) with weight.reshape(K*Fin, F)[fi*K + k]
    W_eff = np.asarray(weight, np.float32).reshape(K * FIN, FOUT) \
        .reshape(FIN, K, FOUT).transpose(1, 0, 2)
    Wblk = np.zeros((K, CW, BFO), np.float32)
    for k in range(K):
        for b in range(B):
            Wblk[k, b * FIN:(b + 1) * FIN, b * FOUT:(b + 1) * FOUT] = W_eff[k]
    Wblk = Wblk.astype(bfloat16)
    bias_row = np.tile(np.tile(np.asarray(bias, np.float32), B)[None, :],
                       (128, 1))
    ident = np.eye(128, dtype=np.float32).astype(bfloat16)
    negi = (-np.eye(128, dtype=np.float32)).astype(bfloat16)
    iorow = np.tile(np.arange(128, dtype=np.float32)[None, :],
                    (128, 1)).astype(bfloat16)
    xf0a = np.ascontiguousarray(
        np.concatenate([x0[c * VC:c * VC + HALF] for c in range(C)]))
    xf0b = np.ascontiguousarray(
        np.concatenate([x0[c * VC + HALF:(c + 1) * VC] for c in range(C)]))
    in_maps = []
    for c in range(C):
        in_maps.append({
            "x0s": x0[c * VC:(c + 1) * VC],
            "xf00": xf0a, "xf01": xf0b,
            "gidx": plan.gidx[c], "dest": plan.dest[c],
            "gvals": plan.gvals[c], "gvals2": plan.gvals2[c],
            "wblk": Wblk, "bias_row": bias_row,
            "iden": ident, "negi": negi, "iorow": iorow,
        })
    return plan, in_maps


# ----------------------------------------------------------------------------
# Device program
# ----------------------------------------------------------------------------

def build_program(plan):
    TOT = plan.TOT
    NGMAX = plan.NGMAX

    nc = bacc.Bacc("TRN2", target_bir_lowering=False, debug=False,
                   num_devices=C)
    x0s = nc.dram_tensor("x0s", [VC, CW], BF16, kind="ExternalInput")
    gidx = nc.dram_tensor("gidx", [128, TOT // 16], I16, kind="ExternalInput")
    dest = nc.dram_tensor("dest", [128, TOT // 128], BF16,
                          kind="ExternalInput")
    gvals = nc.dram_tensor("gvals", [128, TOT // 128], BF16,
                           kind="ExternalInput")
    gvals2 = nc.dram_tensor("gvals2", [128, TOT // 128], BF16,
                            kind="ExternalInput")
    wblk = nc.dram_tensor("wblk", [K, CW, BFO], BF16, kind="ExternalInput")
    bias_row = nc.dram_tensor("bias_row", [128, BFO], F32,
                              kind="ExternalInput")
    iden = nc.dram_tensor("iden", [128, 128], BF16, kind="ExternalInput")
    negi = nc.dram_tensor("negi", [128, 128], BF16, kind="ExternalInput")
    iorow = nc.dram_tensor("iorow", [128, 128], BF16, kind="ExternalInput")
    out = nc.dram_tensor("out", [VC, BFO], F32, kind="ExternalOutput")
    dbg = nc.dram_tensor("dbg", [512, 128], F32, kind="ExternalOutput") \
        if DEBUG_DUMP else None

    # x0 gather buffers are precomputed on host (replicated inputs); the
    # steps' buffers are AllGathered internally.
    xf0 = [nc.dram_tensor(f"xf0{h}", [HALF * C, CW], BF16,
                          kind="ExternalInput") for h in range(2)]
    xf = [xf0] + [[nc.dram_tensor(f"xf{s}_{h}", [HALF * C, CW], BF16,
                                  addr_space="Shared")
                   for h in range(2)] for s in range(1, KS)]
    yb = [[nc.dram_tensor(f"y{s}_{h}", [HALF, CW], BF16) for h in range(2)]
          for s in range(1, KS + 1)]        # yb[s-1] = x_s shard halves

    def shard_rows(s, r0, r1):
        """AP for local shard rows [r0, r1) of x_s (s=0 -> x0s input)."""
        if s == 0:
            return x0s[r0:r1, :]
        src = yb[s - 1]
        assert (r0 < HALF) == (r1 <= HALF)
        if r1 <= HALF:
            return src[0][r0:r1, :]
        return src[1][r0 - HALF:r1 - HALF, :]

    rg_list = [[0, 1, 2, 3, 4, 5, 6, 7]]

    with tile.TileContext(nc) as tc:
        with tc.tile_pool(name="cns", bufs=1) as cns:
            iot = cns.tile([128, 128], BF16, tag="iot")
            nc.sync.dma_start(iot[:], iorow[:])
            ngt = cns.tile([128, 128], BF16, tag="ngt")
            nc.sync.dma_start(ngt[:], negi[:])

            with tc.tile_pool(name="gip", bufs=2) as gip, \
                 tc.tile_pool(name="dtp", bufs=2) as dtp, \
                 tc.tile_pool(name="vtp", bufs=2) as vtp, \
                 tc.tile_pool(name="rng", bufs=3) as rng, \
                 tc.tile_pool(name="smp", bufs=3) as smp, \
                 tc.tile_pool(name="ysp", bufs=2) as ysp, \
                 tc.tile_pool(name="xpp", bufs=2) as xpp, \
                 tc.tile_pool(name="psp", bufs=1, space="PSUM") as psp:
                for s in range(KS):
                    vsrc = gvals if s == 0 else gvals2
                    for st in range(NST):
                        st_o0 = plan.st_groups[st][0][0]
                        st_n = sum(n for (_, n) in plan.st_groups[st])
                        # per-ST index/dest/val loads (one DMA each)
                        gi = gip.tile([128, plan.NST16], I16, tag="gi")
                        nc.sync.dma_start(
                            gi[:, :st_n // 16],
                            gidx[:, st_o0 // 16:(st_o0 + st_n) // 16])
                        dt = dtp.tile([128, plan.NST128], BF16, tag="dt")
                        nc.sync.dma_start(
                            dt[:, :st_n // 128],
                            dest[:, st_o0 // 128:(st_o0 + st_n) // 128])
                        vt = vtp.tile([128, plan.NST128], BF16, tag="vt")
                        nc.sync.dma_start(
                            vt[:, :st_n // 128],
                            vsrc[:, st_o0 // 128:(st_o0 + st_n) // 128])
                        ps = [psp.tile([128, 128], F32, tag=f"ps{q}",
                                       name=f"ps{q}") for q in range(ST_T)]
                        for r in range(NREG):
                            o0, n = plan.st_groups[st][r]
                            if n == 0:
                                continue
                            ng = n // 128
                            lo16 = (o0 - st_o0) // 16
                            lo128 = (o0 - st_o0) // 128
                            rg = rng.tile([128, NGMAX, CW], BF16, tag="rg")
                            src = xf[s][r // 3][(r % 3) * RG:(r % 3 + 1) * RG, :]
                            for p0 in range(0, n, CALL_CAP):
                                pn = min(CALL_CAP, n - p0)
                                nc.gpsimd.dma_gather(
                                    rg[:, p0 // 128:(p0 + pn) // 128, :], src,
                                    gi[:, lo16 + p0 // 16:
                                       lo16 + (p0 + pn) // 16],
                                    pn, pn, CW)
                            sm = smp.tile([128, NGMAX, CW], BF16, tag="sm")
                            nc.vector.tensor_tensor(
                                sm[:, :ng, :],
                                dt[:, lo128:lo128 + ng].unsqueeze(-1)
                                .broadcast_to([128, ng, CW]),
                                iot[:].unsqueeze(1).broadcast_to([128, ng, CW]),
                                mybir.AluOpType.is_equal)
                            nc.vector.tensor_tensor(
                                sm[:, :ng, :], sm[:, :ng, :],
                                vt[:, lo128:lo128 + ng].unsqueeze(-1)
                                .broadcast_to([128, ng, CW]),
                                mybir.AluOpType.mult)
                            if DEBUG_DUMP and s == 0 and st == 0 and r == 0:
                                dts = smp.tile([128, 128], F32, tag="dts",
                                               name="dts")
                                nc.vector.tensor_copy(dts[:], sm[:, 0, :])
                                nc.sync.dma_start(dbg[0:128, :], dts[:])
                                dtr = smp.tile([128, 128], F32, tag="dtr",
                                               name="dtr")
                                nc.vector.tensor_copy(dtr[:], rg[:, 0, :])
                                nc.sync.dma_start(dbg[128:256, :], dtr[:])
                            for (ti, ci, first) in plan.st_mm[st][r]:
                                oap = ps[ti][:, :]
                                stop = (s == 0 and
                                        plan.st_last[st][ti] == (r, ci))
                                nc.tensor.matmul(
                                    oap, sm[:, ci, :], rg[:, ci, :],
                                    start=bool(first), stop=stop)
                        ys = ysp.tile([128, ST_T, CW], BF16, tag="ys")
                        if s > 0:
                            xp = xpp.tile([128, ST_T, CW], BF16, tag="xp")
                            nc.sync.dma_start(
                                xp[:],
                                shard_rows(s - 1, st * 1024, (st + 1) * 1024)
                                .rearrange("(ts p) c -> p ts c", p=128))
                            for ti in range(ST_T):
                                nc.tensor.matmul(ps[ti][:, :], ngt[:],
                                                 xp[:, ti, :],
                                                 start=False, stop=True)
                        for ti in range(ST_T):
                            nc.vector.tensor_copy(ys[:, ti, :], ps[ti][:, :])
                        if DEBUG_DUMP and s == 0 and st == 0:
                            dty = ysp.tile([128, 128], F32, tag="dty",
                                           name="dty")
                            nc.vector.tensor_copy(dty[:], ys[:, 0, :])
                            nc.sync.dma_start(dbg[256:384, :], dty[:])
                        if DEBUG_DUMP and s == 1 and st == 0:
                            dtz = ysp.tile([128, 128], F32, tag="dtz",
                                           name="dtz")
                            nc.vector.tensor_copy(dtz[:], ys[:, 0, :])
                            nc.sync.dma_start(dbg[384:512, :], dtz[:])
                        h, hb = (0, st * 1024) if st < NST // 2 else \
                            (1, st * 1024 - HALF)
                        nc.sync.dma_start(
                            yb[s][h][hb:hb + 1024, :]
                            .rearrange("(ts p) c -> p ts c", p=128), ys[:])
                        if st == NST // 2 - 1 and s < KS - 1:
                            nc.gpsimd.collective_compute(
                                "AllGather", mybir.AluOpType.bypass,
                                replica_groups=rg_list,
                                ins=[yb[s][0].ap().opt()],
                                outs=[xf[s + 1][0].ap().opt()])
                        if st == NST - 1 and s < KS - 1:
                            nc.gpsimd.collective_compute(
                                "AllGather", mybir.AluOpType.bypass,
                                replica_groups=rg_list,
                                ins=[yb[s][1].ap().opt()],
                                outs=[xf[s + 1][1].ap().opt()])

        # ---- projection ----
        VSUP = 2048
        NSUP = VC // VSUP
        TSUB = VSUP // 128
        with tc.tile_pool(name="cwp", bufs=1) as cwp, \
             tc.tile_pool(name="cbp", bufs=2) as cbp, \
             tc.tile_pool(name="pst", bufs=2, space="PSUM") as pst, \
             tc.tile_pool(name="pso", bufs=2, space="PSUM") as pso, \
             tc.tile_pool(name="lhp", bufs=3) as lhp, \
             tc.tile_pool(name="stp", bufs=2) as stp:
            wt = []
            for k in range(K):
                wtk = cwp.tile([CW, BFO], BF16, tag=f"w{k}")
                wt.append(wtk)
            for k in range(K):
                nc.sync.dma_start(wt[k][:], wblk[k, :, :])
            bt = cwp.tile([128, BFO], F32, tag="bias")
            nc.sync.dma_start(bt[:], bias_row[:])
            idt = cwp.tile([128, 128], BF16, tag="ident")
            nc.sync.dma_start(idt[:], iden[:])
            for sc in range(NSUP):
                vbase = sc * VSUP
                cb = []
                for k in range(K):
                    cbt = cbp.tile([128, TSUB, CW], BF16, tag=f"cb{k}")
                    nc.sync.dma_start(
                        cbt[:],
                        shard_rows(k, vbase, vbase + VSUP)
                        .rearrange("(ts p) c -> p ts c", p=128))
                    cb.append(cbt)
                stl = stp.tile([128, TSUB, BFO], F32, tag="stage")
                for ts in range(TSUB):
                    po = pso.tile([128, BFO], F32, tag="po")
                    for k in range(K):
                        pt = pst.tile([128, 128], BF16, tag="pt")
                        nc.tensor.transpose(pt[:, :CW], cb[k][:, ts, :],
                                            idt[:])
                        lh = lhp.tile([128, CW], BF16, tag="lh")
                        nc.vector.tensor_copy(lh[:, :CW], pt[:, :CW])
                        nc.tensor.matmul(po[:], lh[:, :CW], wt[k][:],
                                         start=(k == 0), stop=(k == K - 1))
                    nc.vector.tensor_tensor(
                        stl[:, ts, :], po[:], bt[:], mybir.AluOpType.add)
                nc.sync.dma_start(
                    out[vbase:vbase + VSUP, :]
                    .rearrange("(ts p) c -> p ts c", p=128), stl[:])
    nc.compile()
    return nc


# ----------------------------------------------------------------------------
# Self-contained entry point (full inputs in, full output out)
# ----------------------------------------------------------------------------

_CACHE = {}


def kernel(lap_rows, lap_cols, lap_vals, inputs, weight, bias):
    """ChebConv on 8 TRN2 NeuronCores. Full inputs -> full [B, V, FOUT]."""
    inputs = np.asarray(inputs, np.float32)
    weight = np.asarray(weight, np.float32)
    bias = np.asarray(bias, np.float32)
    plan, in_maps = host_prep(lap_rows, lap_cols, lap_vals, inputs, weight,
                              bias)
    key = ("v2", plan.TOT)
    if key not in _CACHE:
        _CACHE.clear()
        _CACHE[key] = build_program(plan)
    nc = _CACHE[key]
    res = bass_utils.run_bass_kernel_spmd(nc, in_maps, core_ids=list(range(C)))
    outs = []
    for c in range(C):
        o = res.results[c]["out"]
        outs.append(o.reshape(VC, B, FOUT).transpose(1, 0, 2))
    return np.ascontiguousarray(np.concatenate(outs, axis=1)).astype(np.float32)


# revision 4
# speedup vs baseline: 1.0478x; 1.0070x over previous
"""ChebConv SpMM kernel for 8 TRN2 NeuronCores — matmul-aggregation version.

Strategy (dest-sharded graph-parallel):
- x held as [V, 128] bf16 (col = b*32 + fi). Core c owns dest rows
  [c*Vc, (c+1)*Vc). Full x is AllGathered (bf16, half-shard granularity
  for overlap) into per-step gather buffers xfA/xfB with row order
  (core, half-local row).
- Each Chebyshev step, per dest supertile of 16 dest tiles (128 dests
  each): dma_gather the edges' source rows (256B bf16 elements) from the
  6 int16-index regions, build a scaled one-hot matrix
  S[e, d] = val_e * (dest_e == d) on DVE (is_equal vs a host-provided
  iota row matrix, then multiply by vals), and accumulate
  y_tile = sum_chunks S^T-matmul-E into PSUM on the PE. The recurrence
  term -x_{k-1} is folded in as one extra matmul with a -I stationary.
  No dma_scatter_add anywhere.
- Edges are host-sorted into (dest supertile, source region, dest tile)
  order; each (tile, region) group is padded to a multiple of 128 edges
  (val=0 pads) so every matmul chunk is a full 128-edge slab.
- Final projection out = sum_k T_k W_k: PE transpose of bf16 cheb tiles
  + matmul against block-diagonal bf16 W, bias added on the PSUM->SBUF
  copy (f32 out).
"""
import sys

sys.path.insert(0, "/opt/trn_rl_repo")

import numpy as np
from ml_dtypes import bfloat16

import concourse.bass as bass
import concourse.bacc as bacc
import concourse.mybir as mybir
import concourse.tile as tile
from concourse import bass_utils

F32 = mybir.dt.float32
BF16 = mybir.dt.bfloat16
I16 = mybir.dt.int16

DEBUG_DUMP = False
CALL_CAP = 1024        # idxs per dma_gather call (<=2048: 128-deep desc ring)

V = 196608
C = 8
VC = V // C            # 24576 dest rows per core
HALF = VC // 2         # 12288
NT = VC // 128         # 192 dest tiles per core
ST_T = 8               # dest tiles per supertile (= PSUM banks)
NST = NT // ST_T       # 12 supertiles
RG = 32768             # gather region rows (int16 index range)
NREG = 6               # regions over the 2x[98304,128] gather buffers
B, FIN, FOUT, K = 4, 32, 64, 5
CW = B * FIN           # 128
BFO = B * FOUT         # 256
KS = K - 1             # SpMM steps


# ----------------------------------------------------------------------------
# Host-side preprocessing
# ----------------------------------------------------------------------------

class Plan:
    pass


def build_plan(lap_rows, lap_cols, lap_vals):
    rows = np.asarray(lap_rows).astype(np.int64)
    cols = np.asarray(lap_cols).astype(np.int64)
    vals = np.asarray(lap_vals).astype(np.float32)
    nnz = len(rows)

    core = rows // VC
    tloc = (rows % VC) // 128          # dest tile within core
    dest = rows % 128                  # dest within tile
    csrc = cols // VC
    j = cols % VC
    # gather buffer k (k = j//4096) holds rows {c*4096 + j%4096} of every
    # core: exactly one 32768-row int16-indexable region per buffer, so the
    # per-region AllGather that fills it can fire as soon as all cores have
    # finished the 4 supertiles covering those dest rows.
    reg = j // (VC // NREG)            # 0..5
    gidx = csrc * (VC // NREG) + (j % (VC // NREG))

    gk = (core * NT + tloc) * NREG + reg
    order = np.argsort(gk, kind="stable")
    gs = gk[order]
    starts = np.r_[0, np.nonzero(np.diff(gs))[0] + 1]
    grp_start = np.repeat(starts, np.diff(np.r_[starts, nnz]))
    within = np.arange(nnz) - grp_start

    cnt = np.bincount(gk, minlength=C * NT * NREG).reshape(C, NT, NREG)
    mx = cnt.max(axis=0)               # [NT, NREG]
    assert cnt.sum(axis=(0, 2)).min() > 0
    chunks = (mx + 127) // 128         # [NT, NREG]

    # slot layout: (supertile, region, tile) order
    off = np.zeros((NT, NREG), np.int64)
    cursor = 0
    st_groups = []                     # [NST][NREG] -> (o0, n)
    st_mm = []                         # [NST][NREG] -> list of (ti, ci, first)
    st_last = []                       # [NST] -> per-ti (r, ci) of last chunk
    for st in range(NST):
        groups_r = []
        mm_r = []
        tot_ch = chunks[st * ST_T:(st + 1) * ST_T].sum(axis=1)
        done = np.zeros(ST_T, np.int64)
        last = [None] * ST_T
        for r in range(NREG):
            o0 = cursor
            mm = []
            ci = 0
            for ti in range(ST_T):
                t = st * ST_T + ti
                off[t, r] = cursor
                nch = int(chunks[t, r])
                for _ in range(nch):
                    mm.append((ti, ci, done[ti] == 0))
                    done[ti] += 1
                    if done[ti] == tot_ch[ti]:
                        last[ti] = (r, ci)
                    ci += 1
                cursor += nch * 128
            groups_r.append((o0, cursor - o0))
            mm_r.append(mm)
        assert all(l is not None for l in last)
        st_groups.append(groups_r)
        st_mm.append(mm_r)
        st_last.append(last)
    TOT = cursor
    assert TOT % 128 == 0

    t_s = tloc[order]
    r_s = reg[order]
    c_s = core[order]
    slot_s = off[t_s, r_s] + within

    g_arr = np.zeros((C, TOT), np.int16)
    d_arr = np.zeros((C, TOT), np.float32)
    v_arr = np.zeros((C, TOT), np.float32)
    g_arr[c_s, slot_s] = gidx[order].astype(np.int16)
    d_arr[c_s, slot_s] = dest[order]
    v_arr[c_s, slot_s] = vals[order]

    def wrap16(a):                     # [C, TOT] -> [C, 128, TOT//16]
        w = a.reshape(C, TOT // 16, 16).transpose(0, 2, 1)
        return np.tile(w, (1, 8, 1)).copy()

    def wrap128(a):
        return a.reshape(C, TOT // 128, 128).transpose(0, 2, 1).copy()

    plan = Plan()
    plan.TOT = TOT
    plan.st_groups = st_groups
    plan.st_mm = st_mm
    plan.st_last = st_last
    plan.NGMAX = max(n // 128 for g in st_groups for (_, n) in g)
    plan.NST16 = max(sum(n for (_, n) in g) // 16 for g in st_groups)
    plan.NST128 = max(sum(n for (_, n) in g) // 128 for g in st_groups)
    plan.gidx = wrap16(g_arr)
    plan.dest = wrap128(d_arr).astype(bfloat16)
    plan.gvals = wrap128(v_arr).astype(bfloat16)
    plan.gvals2 = wrap128(2.0 * v_arr).astype(bfloat16)
    return plan


def host_prep(lap_rows, lap_cols, lap_vals, inputs, weight, bias):
    plan = build_plan(lap_rows, lap_cols, lap_vals)
    # x0 [V, 128], col = b*FIN + fi
    x0 = np.ascontiguousarray(
        np.asarray(inputs, np.float32).transpose(1, 0, 2).reshape(V, CW)
    ).astype(bfloat16)
    # Reference pairs cheb col (fi, k) with weight.reshape(K*Fin, F)[fi*K + k]
    W_eff = np.asarray(weight, np.float32).reshape(K * FIN, FOUT) \
        .reshape(FIN, K, FOUT).transpose(1, 0, 2)
    Wblk = np.zeros((K, CW, BFO), np.float32)
    for k in range(K):
        for b in range(B):
            Wblk[k, b * FIN:(b + 1) * FIN, b * FOUT:(b + 1) * FOUT] = W_eff[k]
    Wblk = Wblk.astype(bfloat16)
    bias_row = np.tile(np.tile(np.asarray(bias, np.float32), B)[None, :],
                       (128, 1))
    ident = np.eye(128, dtype=np.float32).astype(bfloat16)
    negi = (-np.eye(128, dtype=np.float32)).astype(bfloat16)
    iorow = np.tile(np.arange(128, dtype=np.float32)[None, :],
                    (128, 1)).astype(bfloat16)
    RQ = VC // NREG
    xf0r = [np.ascontiguousarray(np.concatenate(
        [x0[c * VC + k * RQ:c * VC + (k + 1) * RQ] for c in range(C)]))
        for k in range(NREG)]
    in_maps = []
    for c in range(C):
        in_maps.append({
            "x0s": x0[c * VC:(c + 1) * VC],
            **{f"xf0{k}": xf0r[k] for k in range(NREG)},
            "gidx": plan.gidx[c], "dest": plan.dest[c],
            "gvals": plan.gvals[c], "gvals2": plan.gvals2[c],
            "wblk": Wblk, "bias_row": bias_row,
            "iden": ident, "negi": negi, "iorow": iorow,
        })
    return plan, in_maps


# ----------------------------------------------------------------------------
# Device program
# ----------------------------------------------------------------------------

def build_program(plan):
    TOT = plan.TOT
    NGMAX = plan.NGMAX

    nc = bacc.Bacc("TRN2", target_bir_lowering=False, debug=False,
                   num_devices=C)
    x0s = nc.dram_tensor("x0s", [VC, CW], BF16, kind="ExternalInput")
    gidx = nc.dram_tensor("gidx", [128, TOT // 16], I16, kind="ExternalInput")
    dest = nc.dram_tensor("dest", [128, TOT // 128], BF16,
                          kind="ExternalInput")
    gvals = nc.dram_tensor("gvals", [128, TOT // 128], BF16,
                           kind="ExternalInput")
    gvals2 = nc.dram_tensor("gvals2", [128, TOT // 128], BF16,
                            kind="ExternalInput")
    wblk = nc.dram_tensor("wblk", [K, CW, BFO], BF16, kind="ExternalInput")
    bias_row = nc.dram_tensor("bias_row", [128, BFO], F32,
                              kind="ExternalInput")
    iden = nc.dram_tensor("iden", [128, 128], BF16, kind="ExternalInput")
    negi = nc.dram_tensor("negi", [128, 128], BF16, kind="ExternalInput")
    iorow = nc.dram_tensor("iorow", [128, 128], BF16, kind="ExternalInput")
    out = nc.dram_tensor("out", [VC, BFO], F32, kind="ExternalOutput")
    dbg = nc.dram_tensor("dbg", [512, 128], F32, kind="ExternalOutput") \
        if DEBUG_DUMP else None

    # x0 gather region buffers are precomputed on host (replicated inputs);
    # the steps' buffers are AllGathered internally, one AG per region.
    RQ = VC // NREG
    xf0 = [nc.dram_tensor(f"xf0{k}", [RG, CW], BF16, kind="ExternalInput")
           for k in range(NREG)]
    xf = [xf0] + [[nc.dram_tensor(f"xf{s}_{k}", [RG, CW], BF16,
                                  addr_space="Shared")
                   for k in range(NREG)] for s in range(1, KS)]
    yb = [[nc.dram_tensor(f"y{s}_{k}", [RQ, CW], BF16) for k in range(NREG)]
          for s in range(1, KS + 1)]        # yb[s-1][k] = x_s shard region k

    def shard_rows(s, r0, r1):
        """AP for local shard rows [r0, r1) of x_s (s=0 -> x0s input)."""
        if s == 0:
            return x0s[r0:r1, :]
        k = r0 // RQ
        assert r1 <= (k + 1) * RQ
        return yb[s - 1][k][r0 - k * RQ:r1 - k * RQ, :]

    rg_list = [[0, 1, 2, 3, 4, 5, 6, 7]]

    with tile.TileContext(nc) as tc:
        with tc.tile_pool(name="cns", bufs=1) as cns:
            iot = cns.tile([128, 128], BF16, tag="iot")
            nc.sync.dma_start(iot[:], iorow[:])
            ngt = cns.tile([128, 128], BF16, tag="ngt")
            nc.sync.dma_start(ngt[:], negi[:])

            with tc.tile_pool(name="gip", bufs=2) as gip, \
                 tc.tile_pool(name="dtp", bufs=2) as dtp, \
                 tc.tile_pool(name="vtp", bufs=2) as vtp, \
                 tc.tile_pool(name="rng", bufs=3) as rng, \
                 tc.tile_pool(name="smp", bufs=3) as smp, \
                 tc.tile_pool(name="ysp", bufs=2) as ysp, \
                 tc.tile_pool(name="xpp", bufs=2) as xpp, \
                 tc.tile_pool(name="psp", bufs=1, space="PSUM") as psp:
                for s in range(KS):
                    vsrc = gvals if s == 0 else gvals2
                    for st in range(NST):
                        st_o0 = plan.st_groups[st][0][0]
                        st_n = sum(n for (_, n) in plan.st_groups[st])
                        # per-ST index/dest/val loads (one DMA each)
                        gi = gip.tile([128, plan.NST16], I16, tag="gi")
                        nc.sync.dma_start(
                            gi[:, :st_n // 16],
                            gidx[:, st_o0 // 16:(st_o0 + st_n) // 16])
                        dt = dtp.tile([128, plan.NST128], BF16, tag="dt")
                        nc.sync.dma_start(
                            dt[:, :st_n // 128],
                            dest[:, st_o0 // 128:(st_o0 + st_n) // 128])
                        vt = vtp.tile([128, plan.NST128], BF16, tag="vt")
                        nc.sync.dma_start(
                            vt[:, :st_n // 128],
                            vsrc[:, st_o0 // 128:(st_o0 + st_n) // 128])
                        ps = [psp.tile([128, 128], F32, tag=f"ps{q}",
                                       name=f"ps{q}") for q in range(ST_T)]
                        for r in range(NREG):
                            o0, n = plan.st_groups[st][r]
                            if n == 0:
                                continue
                            ng = n // 128
                            lo16 = (o0 - st_o0) // 16
                            lo128 = (o0 - st_o0) // 128
                            rg = rng.tile([128, NGMAX, CW], BF16, tag="rg")
                            src = xf[s][r][:, :]
                            for p0 in range(0, n, CALL_CAP):
                                pn = min(CALL_CAP, n - p0)
                                nc.gpsimd.dma_gather(
                                    rg[:, p0 // 128:(p0 + pn) // 128, :], src,
                                    gi[:, lo16 + p0 // 16:
                                       lo16 + (p0 + pn) // 16],
                                    pn, pn, CW)
                            sm = smp.tile([128, NGMAX, CW], BF16, tag="sm")
                            nc.vector.tensor_tensor(
                                sm[:, :ng, :],
                                dt[:, lo128:lo128 + ng].unsqueeze(-1)
                                .broadcast_to([128, ng, CW]),
                                iot[:].unsqueeze(1).broadcast_to([128, ng, CW]),
                                mybir.AluOpType.is_equal)
                            nc.vector.tensor_tensor(
                                sm[:, :ng, :], sm[:, :ng, :],
                                vt[:, lo128:lo128 + ng].unsqueeze(-1)
                                .broadcast_to([128, ng, CW]),
                                mybir.AluOpType.mult)
                            if DEBUG_DUMP and s == 0 and st == 0 and r == 0:
                                dts = smp.tile([128, 128], F32, tag="dts",
                                               name="dts")
                                nc.vector.tensor_copy(dts[:], sm[:, 0, :])
                                nc.sync.dma_start(dbg[0:128, :], dts[:])
                                dtr = smp.tile([128, 128], F32, tag="dtr",
                                               name="dtr")
                                nc.vector.tensor_copy(dtr[:], rg[:, 0, :])
                                nc.sync.dma_start(dbg[128:256, :], dtr[:])
                            for (ti, ci, first) in plan.st_mm[st][r]:
                                oap = ps[ti][:, :]
                                stop = (s == 0 and
                                        plan.st_last[st][ti] == (r, ci))
                                nc.tensor.matmul(
                                    oap, sm[:, ci, :], rg[:, ci, :],
                                    start=bool(first), stop=stop)
                        ys = ysp.tile([128, ST_T, CW], BF16, tag="ys")
                        if s > 0:
                            xp = xpp.tile([128, ST_T, CW], BF16, tag="xp")
                            nc.sync.dma_start(
                                xp[:],
                                shard_rows(s - 1, st * 1024, (st + 1) * 1024)
                                .rearrange("(ts p) c -> p ts c", p=128))
                            for ti in range(ST_T):
                                nc.tensor.matmul(ps[ti][:, :], ngt[:],
                                                 xp[:, ti, :],
                                                 start=False, stop=True)
                        for ti in range(ST_T):
                            nc.vector.tensor_copy(ys[:, ti, :], ps[ti][:, :])
                        if DEBUG_DUMP and s == 0 and st == 0:
                            dty = ysp.tile([128, 128], F32, tag="dty",
                                           name="dty")
                            nc.vector.tensor_copy(dty[:], ys[:, 0, :])
                            nc.sync.dma_start(dbg[256:384, :], dty[:])
                        if DEBUG_DUMP and s == 1 and st == 0:
                            dtz = ysp.tile([128, 128], F32, tag="dtz",
                                           name="dtz")
                            nc.vector.tensor_copy(dtz[:], ys[:, 0, :])
                            nc.sync.dma_start(dbg[384:512, :], dtz[:])
                        k, kb = st // 4, (st % 4) * 1024
                        nc.sync.dma_start(
                            yb[s][k][kb:kb + 1024, :]
                            .rearrange("(ts p) c -> p ts c", p=128), ys[:])
                        if st % 4 == 3 and s < KS - 1:
                            nc.gpsimd.collective_compute(
                                "AllGather", mybir.AluOpType.bypass,
                                replica_groups=rg_list,
                                ins=[yb[s][k].ap().opt()],
                                outs=[xf[s + 1][k].ap().opt()])

        # ---- projection ----
        VSUP = 2048
        NSUP = VC // VSUP
        TSUB = VSUP // 128
        with tc.tile_pool(name="cwp", bufs=1) as cwp, \
             tc.tile_pool(name="cbp", bufs=2) as cbp, \
             tc.tile_pool(name="pst", bufs=2, space="PSUM") as pst, \
             tc.tile_pool(name="pso", bufs=2, space="PSUM") as pso, \
             tc.tile_pool(name="lhp", bufs=3) as lhp, \
             tc.tile_pool(name="stp", bufs=2) as stp:
            wt = []
            for k in range(K):
                wtk = cwp.tile([CW, BFO], BF16, tag=f"w{k}")
                wt.append(wtk)
            for k in range(K):
                nc.sync.dma_start(wt[k][:], wblk[k, :, :])
            bt = cwp.tile([128, BFO], F32, tag="bias")
            nc.sync.dma_start(bt[:], bias_row[:])
            idt = cwp.tile([128, 128], BF16, tag="ident")
            nc.sync.dma_start(idt[:], iden[:])
            for sc in range(NSUP):
                vbase = sc * VSUP
                cb = []
                for k in range(K):
                    cbt = cbp.tile([128, TSUB, CW], BF16, tag=f"cb{k}")
                    nc.sync.dma_start(
                        cbt[:],
                        shard_rows(k, vbase, vbase + VSUP)
                        .rearrange("(ts p) c -> p ts c", p=128))
                    cb.append(cbt)
                stl = stp.tile([128, TSUB, BFO], F32, tag="stage")
                for ts in range(TSUB):
                    po = pso.tile([128, BFO], F32, tag="po")
                    for k in range(K):
                        pt = pst.tile([128, 128], BF16, tag="pt")
                        nc.tensor.transpose(pt[:, :CW], cb[k][:, ts, :],
                                            idt[:])
                        lh = lhp.tile([128, CW], BF16, tag="lh")
                        nc.vector.tensor_copy(lh[:, :CW], pt[:, :CW])
                        nc.tensor.matmul(po[:], lh[:, :CW], wt[k][:],
                                         start=(k == 0), stop=(k == K - 1))
                    nc.vector.tensor_tensor(
                        stl[:, ts, :], po[:], bt[:], mybir.AluOpType.add)
                nc.sync.dma_start(
                    out[vbase:vbase + VSUP, :]
                    .rearrange("(ts p) c -> p ts c", p=128), stl[:])
    nc.compile()
    return nc


# ----------------------------------------------------------------------------
# Self-contained entry point (full inputs in, full output out)
# ----------------------------------------------------------------------------

_CACHE = {}


def kernel(lap_rows, lap_cols, lap_vals, inputs, weight, bias):
    """ChebConv on 8 TRN2 NeuronCores. Full inputs -> full [B, V, FOUT]."""
    inputs = np.asarray(inputs, np.float32)
    weight = np.asarray(weight, np.float32)
    bias = np.asarray(bias, np.float32)
    plan, in_maps = host_prep(lap_rows, lap_cols, lap_vals, inputs, weight,
                              bias)
    key = ("v2", plan.TOT)
    if key not in _CACHE:
        _CACHE.clear()
        _CACHE[key] = build_program(plan)
    nc = _CACHE[key]
    res = bass_utils.run_bass_kernel_spmd(nc, in_maps, core_ids=list(range(C)))
    outs = []
    for c in range(C):
        o = res.results[c]["out"]
        outs.append(o.reshape(VC, B, FOUT).transpose(1, 0, 2))
    return np.ascontiguousarray(np.concatenate(outs, axis=1)).astype(np.float32)


# revision 5
# speedup vs baseline: 1.0669x; 1.0182x over previous
"""ChebConv SpMM kernel for 8 TRN2 NeuronCores — matmul-aggregation version.

Strategy (dest-sharded graph-parallel):
- x held as [V, 128] bf16 (col = b*32 + fi). Core c owns dest rows
  [c*Vc, (c+1)*Vc). Full x is AllGathered (bf16, half-shard granularity
  for overlap) into per-step gather buffers xfA/xfB with row order
  (core, half-local row).
- Each Chebyshev step, per dest supertile of 16 dest tiles (128 dests
  each): dma_gather the edges' source rows (256B bf16 elements) from the
  6 int16-index regions, build a scaled one-hot matrix
  S[e, d] = val_e * (dest_e == d) on DVE (is_equal vs a host-provided
  iota row matrix, then multiply by vals), and accumulate
  y_tile = sum_chunks S^T-matmul-E into PSUM on the PE. The recurrence
  term -x_{k-1} is folded in as one extra matmul with a -I stationary.
  No dma_scatter_add anywhere.
- Edges are host-sorted into (dest supertile, source region, dest tile)
  order; each (tile, region) group is padded to a multiple of 128 edges
  (val=0 pads) so every matmul chunk is a full 128-edge slab.
- Final projection out = sum_k T_k W_k: PE transpose of bf16 cheb tiles
  + matmul against block-diagonal bf16 W, bias added on the PSUM->SBUF
  copy (f32 out).
"""
import sys

sys.path.insert(0, "/opt/trn_rl_repo")

import numpy as np
from ml_dtypes import bfloat16

import concourse.bass as bass
import concourse.bacc as bacc
import concourse.mybir as mybir
import concourse.tile as tile
from concourse import bass_utils

F32 = mybir.dt.float32
BF16 = mybir.dt.bfloat16
I16 = mybir.dt.int16

DEBUG_DUMP = False
CALL_CAP = 1024        # idxs per dma_gather call (<=2048: 128-deep desc ring)

V = 196608
C = 8
VC = V // C            # 24576 dest rows per core
HALF = VC // 2         # 12288
NT = VC // 128         # 192 dest tiles per core
ST_T = 8               # dest tiles per supertile (= PSUM banks)
NST = NT // ST_T       # 12 supertiles
RG = 32768             # gather region rows (int16 index range)
NREG = 6               # regions over the 2x[98304,128] gather buffers
B, FIN, FOUT, K = 4, 32, 64, 5
CW = B * FIN           # 128
BFO = B * FOUT         # 256
KS = K - 1             # SpMM steps


# ----------------------------------------------------------------------------
# Host-side preprocessing
# ----------------------------------------------------------------------------

class Plan:
    pass


def build_plan(lap_rows, lap_cols, lap_vals):
    rows = np.asarray(lap_rows).astype(np.int64)
    cols = np.asarray(lap_cols).astype(np.int64)
    vals = np.asarray(lap_vals).astype(np.float32)
    nnz = len(rows)

    core = rows // VC
    tloc = (rows % VC) // 128          # dest tile within core
    dest = rows % 128                  # dest within tile
    csrc = cols // VC
    j = cols % VC
    # gather buffer k (k = j//4096) holds rows {c*4096 + j%4096} of every
    # core: exactly one 32768-row int16-indexable region per buffer, so the
    # per-region AllGather that fills it can fire as soon as all cores have
    # finished the 4 supertiles covering those dest rows.
    reg = j // (VC // NREG)            # 0..5
    gidx = csrc * (VC // NREG) + (j % (VC // NREG))

    gk = (core * NT + tloc) * NREG + reg
    order = np.argsort(gk, kind="stable")
    gs = gk[order]
    starts = np.r_[0, np.nonzero(np.diff(gs))[0] + 1]
    grp_start = np.repeat(starts, np.diff(np.r_[starts, nnz]))
    within = np.arange(nnz) - grp_start

    cnt = np.bincount(gk, minlength=C * NT * NREG).reshape(C, NT, NREG)
    mx = cnt.max(axis=0)               # [NT, NREG]
    assert cnt.sum(axis=(0, 2)).min() > 0
    chunks = (mx + 127) // 128         # [NT, NREG]

    # slot layout: (supertile, region, tile) order
    off = np.zeros((NT, NREG), np.int64)
    cursor = 0
    st_groups = []                     # [NST][NREG] -> (o0, n)
    st_mm = []                         # [NST][NREG] -> list of (ti, ci, first)
    st_last = []                       # [NST] -> per-ti (r, ci) of last chunk
    for st in range(NST):
        groups_r = []
        mm_r = []
        tot_ch = chunks[st * ST_T:(st + 1) * ST_T].sum(axis=1)
        done = np.zeros(ST_T, np.int64)
        last = [None] * ST_T
        for r in range(NREG):
            o0 = cursor
            mm = []
            ci = 0
            for ti in range(ST_T):
                t = st * ST_T + ti
                off[t, r] = cursor
                nch = int(chunks[t, r])
                for _ in range(nch):
                    mm.append((ti, ci, done[ti] == 0))
                    done[ti] += 1
                    if done[ti] == tot_ch[ti]:
                        last[ti] = (r, ci)
                    ci += 1
                cursor += nch * 128
            groups_r.append((o0, cursor - o0))
            mm_r.append(mm)
        assert all(l is not None for l in last)
        st_groups.append(groups_r)
        st_mm.append(mm_r)
        st_last.append(last)
    TOT = cursor
    assert TOT % 128 == 0

    t_s = tloc[order]
    r_s = reg[order]
    c_s = core[order]
    slot_s = off[t_s, r_s] + within

    g_arr = np.zeros((C, TOT), np.int16)
    d_arr = np.zeros((C, TOT), np.float32)
    v_arr = np.zeros((C, TOT), np.float32)
    g_arr[c_s, slot_s] = gidx[order].astype(np.int16)
    d_arr[c_s, slot_s] = dest[order]
    v_arr[c_s, slot_s] = vals[order]

    def wrap16(a):                     # [C, TOT] -> [C, 128, TOT//16]
        w = a.reshape(C, TOT // 16, 16).transpose(0, 2, 1)
        return np.tile(w, (1, 8, 1)).copy()

    def wrap128(a):
        return a.reshape(C, TOT // 128, 128).transpose(0, 2, 1).copy()

    plan = Plan()
    plan.TOT = TOT
    plan.st_groups = st_groups
    plan.st_mm = st_mm
    plan.st_last = st_last
    plan.NGMAX = max(n // 128 for g in st_groups for (_, n) in g)
    plan.NST16 = max(sum(n for (_, n) in g) // 16 for g in st_groups)
    plan.NST128 = max(sum(n for (_, n) in g) // 128 for g in st_groups)
    plan.gidx = wrap16(g_arr)
    plan.dest = wrap128(d_arr).astype(bfloat16)
    plan.gvals = wrap128(v_arr).astype(bfloat16)
    plan.gvals2 = wrap128(2.0 * v_arr).astype(bfloat16)
    return plan


def host_prep(lap_rows, lap_cols, lap_vals, inputs, weight, bias):
    plan = build_plan(lap_rows, lap_cols, lap_vals)
    # x0 [V, 128], col = b*FIN + fi
    x0 = np.ascontiguousarray(
        np.asarray(inputs, np.float32).transpose(1, 0, 2).reshape(V, CW)
    ).astype(bfloat16)
    # Reference pairs cheb col (fi, k) with weight.reshape(K*Fin, F)[fi*K + k]
    W_eff = np.asarray(weight, np.float32).reshape(K * FIN, FOUT) \
        .reshape(FIN, K, FOUT).transpose(1, 0, 2)
    Wblk = np.zeros((K, CW, BFO), np.float32)
    for k in range(K):
        for b in range(B):
            Wblk[k, b * FIN:(b + 1) * FIN, b * FOUT:(b + 1) * FOUT] = W_eff[k]
    Wblk = Wblk.astype(bfloat16)
    bias_row = np.tile(np.tile(np.asarray(bias, np.float32), B)[None, :],
                       (128, 1))
    ident = np.eye(128, dtype=np.float32).astype(bfloat16)
    negi = (-np.eye(128, dtype=np.float32)).astype(bfloat16)
    iorow = np.tile(np.arange(128, dtype=np.float32)[None, :],
                    (128, 1)).astype(bfloat16)
    RQ = VC // NREG
    xf0r = [np.ascontiguousarray(np.concatenate(
        [x0[c * VC + k * RQ:c * VC + (k + 1) * RQ] for c in range(C)]))
        for k in range(NREG)]
    in_maps = []
    for c in range(C):
        in_maps.append({
            "x0s": x0[c * VC:(c + 1) * VC],
            **{f"xf0{k}": xf0r[k] for k in range(NREG)},
            "gidx": plan.gidx[c], "dest": plan.dest[c],
            "gvals": plan.gvals[c], "gvals2": plan.gvals2[c],
            "wblk": Wblk, "bias_row": bias_row,
            "iden": ident, "negi": negi, "iorow": iorow,
        })
    return plan, in_maps


# ----------------------------------------------------------------------------
# Device program
# ----------------------------------------------------------------------------

def build_program(plan):
    TOT = plan.TOT
    NGMAX = plan.NGMAX

    nc = bacc.Bacc("TRN2", target_bir_lowering=False, debug=False,
                   num_devices=C)
    x0s = nc.dram_tensor("x0s", [VC, CW], BF16, kind="ExternalInput")
    gidx = nc.dram_tensor("gidx", [128, TOT // 16], I16, kind="ExternalInput")
    dest = nc.dram_tensor("dest", [128, TOT // 128], BF16,
                          kind="ExternalInput")
    gvals = nc.dram_tensor("gvals", [128, TOT // 128], BF16,
                           kind="ExternalInput")
    gvals2 = nc.dram_tensor("gvals2", [128, TOT // 128], BF16,
                            kind="ExternalInput")
    wblk = nc.dram_tensor("wblk", [K, CW, BFO], BF16, kind="ExternalInput")
    bias_row = nc.dram_tensor("bias_row", [128, BFO], F32,
                              kind="ExternalInput")
    iden = nc.dram_tensor("iden", [128, 128], BF16, kind="ExternalInput")
    negi = nc.dram_tensor("negi", [128, 128], BF16, kind="ExternalInput")
    iorow = nc.dram_tensor("iorow", [128, 128], BF16, kind="ExternalInput")
    out = nc.dram_tensor("out", [VC, BFO], F32, kind="ExternalOutput")
    dbg = nc.dram_tensor("dbg", [512, 128], F32, kind="ExternalOutput") \
        if DEBUG_DUMP else None

    # x0 gather region buffers are precomputed on host (replicated inputs);
    # the steps' buffers are AllGathered internally, one AG per region.
    RQ = VC // NREG
    xf0 = [nc.dram_tensor(f"xf0{k}", [RG, CW], BF16, kind="ExternalInput")
           for k in range(NREG)]
    xf = [xf0] + [[nc.dram_tensor(f"xf{s}_{k}", [RG, CW], BF16,
                                  addr_space="Shared")
                   for k in range(NREG)] for s in range(1, KS)]
    yb = [[nc.dram_tensor(f"y{s}_{k}", [RQ, CW], BF16) for k in range(NREG)]
          for s in range(1, KS + 1)]        # yb[s-1][k] = x_s shard region k

    def shard_rows(s, r0, r1):
        """AP for local shard rows [r0, r1) of x_s (s=0 -> x0s input)."""
        if s == 0:
            return x0s[r0:r1, :]
        k = r0 // RQ
        assert r1 <= (k + 1) * RQ
        return yb[s - 1][k][r0 - k * RQ:r1 - k * RQ, :]

    rg_list = [[0, 1, 2, 3, 4, 5, 6, 7]]

    with tile.TileContext(nc) as tc:
        with tc.tile_pool(name="cns", bufs=1) as cns:
            iot = cns.tile([128, 128], BF16, tag="iot")
            nc.sync.dma_start(iot[:], iorow[:])
            ngt = cns.tile([128, 128], BF16, tag="ngt")
            nc.sync.dma_start(ngt[:], negi[:])

            with tc.tile_pool(name="gip", bufs=3) as gip, \
                 tc.tile_pool(name="dtp", bufs=3) as dtp, \
                 tc.tile_pool(name="vtp", bufs=3) as vtp, \
                 tc.tile_pool(name="rng", bufs=6) as rng, \
                 tc.tile_pool(name="smp", bufs=6) as smp, \
                 tc.tile_pool(name="ysp", bufs=3) as ysp, \
                 tc.tile_pool(name="xpp", bufs=3) as xpp, \
                 tc.tile_pool(name="psp", bufs=1, space="PSUM") as psp:
                for s in range(KS):
                    vsrc = gvals if s == 0 else gvals2
                    for st in range(NST):
                        st_o0 = plan.st_groups[st][0][0]
                        st_n = sum(n for (_, n) in plan.st_groups[st])
                        # per-ST index/dest/val loads (one DMA each)
                        gi = gip.tile([128, plan.NST16], I16, tag="gi")
                        nc.sync.dma_start(
                            gi[:, :st_n // 16],
                            gidx[:, st_o0 // 16:(st_o0 + st_n) // 16])
                        dt = dtp.tile([128, plan.NST128], BF16, tag="dt")
                        nc.sync.dma_start(
                            dt[:, :st_n // 128],
                            dest[:, st_o0 // 128:(st_o0 + st_n) // 128])
                        vt = vtp.tile([128, plan.NST128], BF16, tag="vt")
                        nc.sync.dma_start(
                            vt[:, :st_n // 128],
                            vsrc[:, st_o0 // 128:(st_o0 + st_n) // 128])
                        ps = [psp.tile([128, 128], F32, tag=f"ps{q}",
                                       name=f"ps{q}") for q in range(ST_T)]
                        for r in range(NREG):
                            o0, n = plan.st_groups[st][r]
                            if n == 0:
                                continue
                            ng = n // 128
                            lo16 = (o0 - st_o0) // 16
                            lo128 = (o0 - st_o0) // 128
                            rg = rng.tile([128, NGMAX, CW], BF16, tag="rg")
                            src = xf[s][r][:, :]
                            for p0 in range(0, n, CALL_CAP):
                                pn = min(CALL_CAP, n - p0)
                                nc.gpsimd.dma_gather(
                                    rg[:, p0 // 128:(p0 + pn) // 128, :], src,
                                    gi[:, lo16 + p0 // 16:
                                       lo16 + (p0 + pn) // 16],
                                    pn, pn, CW)
                            sm = smp.tile([128, NGMAX, CW], BF16, tag="sm")
                            nc.vector.tensor_tensor(
                                sm[:, :ng, :],
                                dt[:, lo128:lo128 + ng].unsqueeze(-1)
                                .broadcast_to([128, ng, CW]),
                                iot[:].unsqueeze(1).broadcast_to([128, ng, CW]),
                                mybir.AluOpType.is_equal)
                            nc.vector.tensor_tensor(
                                sm[:, :ng, :], sm[:, :ng, :],
                                vt[:, lo128:lo128 + ng].unsqueeze(-1)
                                .broadcast_to([128, ng, CW]),
                                mybir.AluOpType.mult)
                            if DEBUG_DUMP and s == 0 and st == 0 and r == 0:
                                dts = smp.tile([128, 128], F32, tag="dts",
                                               name="dts")
                                nc.vector.tensor_copy(dts[:], sm[:, 0, :])
                                nc.sync.dma_start(dbg[0:128, :], dts[:])
                                dtr = smp.tile([128, 128], F32, tag="dtr",
                                               name="dtr")
                                nc.vector.tensor_copy(dtr[:], rg[:, 0, :])
                                nc.sync.dma_start(dbg[128:256, :], dtr[:])
                            for (ti, ci, first) in plan.st_mm[st][r]:
                                oap = ps[ti][:, :]
                                stop = (s == 0 and
                                        plan.st_last[st][ti] == (r, ci))
                                nc.tensor.matmul(
                                    oap, sm[:, ci, :], rg[:, ci, :],
                                    start=bool(first), stop=stop)
                        ys = ysp.tile([128, ST_T, CW], BF16, tag="ys")
                        if s > 0:
                            xp = xpp.tile([128, ST_T, CW], BF16, tag="xp")
                            nc.sync.dma_start(
                                xp[:],
                                shard_rows(s - 1, st * 1024, (st + 1) * 1024)
                                .rearrange("(ts p) c -> p ts c", p=128))
                            for ti in range(ST_T):
                                nc.tensor.matmul(ps[ti][:, :], ngt[:],
                                                 xp[:, ti, :],
                                                 start=False, stop=True)
                        for ti in range(ST_T):
                            nc.vector.tensor_copy(ys[:, ti, :], ps[ti][:, :])
                        if DEBUG_DUMP and s == 0 and st == 0:
                            dty = ysp.tile([128, 128], F32, tag="dty",
                                           name="dty")
                            nc.vector.tensor_copy(dty[:], ys[:, 0, :])
                            nc.sync.dma_start(dbg[256:384, :], dty[:])
                        if DEBUG_DUMP and s == 1 and st == 0:
                            dtz = ysp.tile([128, 128], F32, tag="dtz",
                                           name="dtz")
                            nc.vector.tensor_copy(dtz[:], ys[:, 0, :])
                            nc.sync.dma_start(dbg[384:512, :], dtz[:])
                        k, kb = st // 4, (st % 4) * 1024
                        nc.sync.dma_start(
                            yb[s][k][kb:kb + 1024, :]
                            .rearrange("(ts p) c -> p ts c", p=128), ys[:])
                        if st % 4 == 3 and s < KS - 1:
                            nc.gpsimd.collective_compute(
                                "AllGather", mybir.AluOpType.bypass,
                                replica_groups=rg_list,
                                ins=[yb[s][k].ap().opt()],
                                outs=[xf[s + 1][k].ap().opt()])

        # ---- projection ----
        VSUP = 2048
        NSUP = VC // VSUP
        TSUB = VSUP // 128
        with tc.tile_pool(name="cwp", bufs=1) as cwp, \
             tc.tile_pool(name="cbp", bufs=2) as cbp, \
             tc.tile_pool(name="pst", bufs=2, space="PSUM") as pst, \
             tc.tile_pool(name="pso", bufs=2, space="PSUM") as pso, \
             tc.tile_pool(name="lhp", bufs=3) as lhp, \
             tc.tile_pool(name="stp", bufs=2) as stp:
            wt = []
            for k in range(K):
                wtk = cwp.tile([CW, BFO], BF16, tag=f"w{k}")
                wt.append(wtk)
            for k in range(K):
                nc.sync.dma_start(wt[k][:], wblk[k, :, :])
            bt = cwp.tile([128, BFO], F32, tag="bias")
            nc.sync.dma_start(bt[:], bias_row[:])
            idt = cwp.tile([128, 128], BF16, tag="ident")
            nc.sync.dma_start(idt[:], iden[:])
            for sc in range(NSUP):
                vbase = sc * VSUP
                cb = []
                for k in range(K):
                    cbt = cbp.tile([128, TSUB, CW], BF16, tag=f"cb{k}")
                    nc.sync.dma_start(
                        cbt[:],
                        shard_rows(k, vbase, vbase + VSUP)
                        .rearrange("(ts p) c -> p ts c", p=128))
                    cb.append(cbt)
                stl = stp.tile([128, TSUB, BFO], F32, tag="stage")
                for ts in range(TSUB):
                    po = pso.tile([128, BFO], F32, tag="po")
                    for k in range(K):
                        pt = pst.tile([128, 128], BF16, tag="pt")
                        nc.tensor.transpose(pt[:, :CW], cb[k][:, ts, :],
                                            idt[:])
                        lh = lhp.tile([128, CW], BF16, tag="lh")
                        nc.vector.tensor_copy(lh[:, :CW], pt[:, :CW])
                        nc.tensor.matmul(po[:], lh[:, :CW], wt[k][:],
                                         start=(k == 0), stop=(k == K - 1))
                    nc.vector.tensor_tensor(
                        stl[:, ts, :], po[:], bt[:], mybir.AluOpType.add)
                nc.sync.dma_start(
                    out[vbase:vbase + VSUP, :]
                    .rearrange("(ts p) c -> p ts c", p=128), stl[:])
    nc.compile()
    return nc


# ----------------------------------------------------------------------------
# Self-contained entry point (full inputs in, full output out)
# ----------------------------------------------------------------------------

_CACHE = {}


def kernel(lap_rows, lap_cols, lap_vals, inputs, weight, bias):
    """ChebConv on 8 TRN2 NeuronCores. Full inputs -> full [B, V, FOUT]."""
    inputs = np.asarray(inputs, np.float32)
    weight = np.asarray(weight, np.float32)
    bias = np.asarray(bias, np.float32)
    plan, in_maps = host_prep(lap_rows, lap_cols, lap_vals, inputs, weight,
                              bias)
    key = ("v2", plan.TOT)
    if key not in _CACHE:
        _CACHE.clear()
        _CACHE[key] = build_program(plan)
    nc = _CACHE[key]
    res = bass_utils.run_bass_kernel_spmd(nc, in_maps, core_ids=list(range(C)))
    outs = []
    for c in range(C):
        o = res.results[c]["out"]
        outs.append(o.reshape(VC, B, FOUT).transpose(1, 0, 2))
    return np.ascontiguousarray(np.concatenate(outs, axis=1)).astype(np.float32)
